# revision 10
# baseline (speedup 1.0000x reference)
"""Trainium2 Bass kernel for the bbox corner-chamfer loss.

Math: the reference builds the 8 corners of each source/target box, forms
the per-box 8x8 squared-distance matrix, takes the min over target corners
and means everything.  In the target box's local frame the target corners
are the axis-aligned set {+-hx} x {+-hy} x {+-hz}, so the min over the 8
target corners separates per coordinate and the 8x8 matrix disappears:

    loss = mean_{n,k} sum_d (|y_k,d| - |h_t,d|/2)^2,   y_k = R_t^T(S_k - c_t)

Mapping (data-parallel over boxes: 8 cores x 128 partitions x 256 lanes,
one [128,1] fp32 partial sum per core, host sums and divides by 8N):
  - plane-major packing [P, K, NB]: every per-angle / per-dim scalar is a
    dense [P, NB] plane (innermost stride 1), so ops qualify for the DVE
    2x_1P perf mode
  - bf16 compute for the rotation / corner stages (fp32 trig inputs and
    fp32 accumulation), doubling DVE throughput; the loss is a mean of 2M
    terms so the bf16 rounding bias is ~1e-4 relative
"""

import numpy as np

N_TOTAL = 262144
N_CORES = 8
P = 128
NB = N_TOTAL // N_CORES // P  # 256 boxes per partition
PI = float(np.pi)

_CACHE = {}


def _build_nc():
    from contextlib import ExitStack

    from concourse import bacc, tile, mybir

    F32 = mybir.dt.float32
    BF16 = mybir.dt.bfloat16
    ALU = mybir.AluOpType
    SIN = mybir.ActivationFunctionType.Sin
    SQ = mybir.ActivationFunctionType.Square
    ABSF = mybir.ActivationFunctionType.Abs

    nc = bacc.Bacc(None)
    src = nc.declare_dram_parameter("source", [P * NB, 9], F32, isOutput=False)
    tgt = nc.declare_dram_parameter("target", [P * NB, 9], F32, isOutput=False)
    # [1,4] row: a [128,1] output DMA is 128 four-byte descriptors whose
    # completion semaphore takes ~6us; reducing across partitions on the
    # (otherwise idle) PE makes the output DMA a single descriptor
    outp = nc.declare_dram_parameter("partial", [1, 4], F32, isOutput=True)

    with tile.TileContext(nc) as tc:
        with ExitStack() as ctx:
            persist = ctx.enter_context(tc.tile_pool(name="persist", bufs=1))
            work1 = ctx.enter_context(tc.tile_pool(name="work1", bufs=8))
            tailp = ctx.enter_context(tc.tile_pool(name="tailp", bufs=2))

            V, A = nc.vector, nc.scalar

            def T(tag, shape=(P, NB), dt=BF16):
                return persist.tile(list(shape), dt, tag=tag, name=tag)[:]

            def TW(shape=(P, NB)):
                return work1.tile(list(shape), BF16, tag="w1", name="w1")[:]

            IN_S = T("in_s", (P, NB, 9), F32)
            IN_T = T("in_t", (P, NB, 9), F32)
            sv = src[:].rearrange("(p b) f -> p b f", p=P)
            tv = tgt[:].rearrange("(p b) f -> p b f", p=P)
            CH = NB // 4
            for q in range(4):
                sl = slice(q * CH, (q + 1) * CH)
                nc.sync.dma_start(out=IN_S[:, sl, :], in_=sv[:, sl, :])
                nc.sync.dma_start(out=IN_T[:, sl, :], in_=tv[:, sl, :])

            # one tiny op per DMA chunk so no later instruction needs more
            # than one DMA-semaphore wait (ISA wait-slot limits)
            touch = T("touch", (P, 8), F32)
            for q in range(4):
                i0 = q * CH
                V.tensor_tensor(touch[:, 2 * q : 2 * q + 1], IN_S[:, i0 : i0 + 1, 0], IN_S[:, i0 : i0 + 1, 0], ALU.bypass)
                V.tensor_tensor(touch[:, 2 * q + 1 : 2 * q + 2], IN_T[:, i0 : i0 + 1, 0], IN_T[:, i0 : i0 + 1, 0], ALU.bypass)

            # --- trig: sin/cos of (sa0-ta0, sa1, sa2, ta0, ta1, ta2) ---
            # per-angle wraps (strided field in, dense plane out, fp32),
            # then one Sin per 6-angle pack; ACT writes bf16 planes
            d0 = T("d0", (P, NB), F32)
            V.tensor_sub(d0, IN_S[:, :, 6], IN_T[:, :, 6])
            SINP = persist.tile([P, 6, NB], F32, tag="trigin", name="sinp")[:]
            COSP = persist.tile([P, 6, NB], F32, tag="trigin", name="cosp")[:]
            sa12 = IN_S[:, :, 7:9].transpose([0, 2, 1])  # [P, 2, NB]
            ta012 = IN_T[:, :, 6:9].transpose([0, 2, 1])  # [P, 3, NB]
            for dst, shift in ((SINP, 0.0), (COSP, PI / 2)):
                V.add_range_wrap(dst[:, 0, :], d0, shift, PI, 2 * PI)
                V.add_range_wrap(dst[:, 1:3, :], sa12, shift, PI, 2 * PI)
                V.add_range_wrap(dst[:, 3:6, :], ta012, shift, PI, 2 * PI)
            S6 = T("s6", (P, 6, NB))
            C6 = T("c6", (P, 6, NB))
            A.activation(S6, SINP, SIN)
            A.activation(C6, COSP, SIN)

            sd0, ss1, ss2, st0, st1, st2 = (S6[:, i, :] for i in range(6))
            cd0, cs1, cs2, ct0, ct1, ct2 = (C6[:, i, :] for i in range(6))

            # --- R = Rz(d0) Rx(sa1) Ry(sa2): rows [P, 3, NB], true signs ---
            R0 = T("R0", (P, 3, NB))
            R1 = T("R1", (P, 3, NB))
            R2 = T("R2", (P, 3, NB))
            x12 = TW()
            V.tensor_mul(x12, ss1, ss2)
            x1c2 = TW()
            V.tensor_mul(x1c2, ss1, cs2)

            def combine(dst, a0, a1, b0, b1, op):
                # dst = a0*a1 (op) b0*b1
                p0, p1 = TW(), TW()
                V.tensor_mul(p0, a0, a1)
                V.tensor_mul(p1, b0, b1)
                V.tensor_tensor(dst, p0, p1, op)

            combine(R0[:, 0, :], cd0, cs2, sd0, x12, ALU.subtract)
            V.scalar_tensor_tensor(R0[:, 1, :], sd0, -1.0, cs1, ALU.mult, ALU.mult)
            combine(R0[:, 2, :], cd0, ss2, sd0, x1c2, ALU.add)
            combine(R1[:, 0, :], sd0, cs2, cd0, x12, ALU.add)
            V.tensor_mul(R1[:, 1, :], cd0, cs1)
            combine(R1[:, 2, :], sd0, ss2, cd0, x1c2, ALU.subtract)
            V.scalar_tensor_tensor(R2[:, 0, :], cs1, -1.0, ss2, ALU.mult, ALU.mult)
            V.tensor_copy(R2[:, 1, :], ss1)
            V.tensor_mul(R2[:, 2, :], cs1, cs2)

            # --- W = Ry(-ta2) Rx(-ta1) R  (rows [P,3,NB], broadcast trig) ---
            def b3(ap1):  # [P,NB] -> [P,3,NB]: stride-0 middle, dense inner
                return ap1.unsqueeze(1).broadcast_to([P, 3, NB])

            def row_combine(dst, c, ra, s, rb, op):
                p0, p1 = TW((P, 3, NB)), TW((P, 3, NB))
                V.tensor_mul(p0, ra, b3(c))
                V.tensor_mul(p1, rb, b3(s))
                V.tensor_tensor(dst, p0, p1, op)

            A1 = T("A1", (P, 3, NB))
            A2 = T("A2", (P, 3, NB))
            row_combine(A1, ct1, R1, st1, R2, ALU.add)
            row_combine(A2, ct1, R2, st1, R1, ALU.subtract)
            W0 = T("W0", (P, 3, NB))
            W2 = T("W2", (P, 3, NB))
            row_combine(W0, ct2, R0, st2, A2, ALU.subtract)
            row_combine(W2, st2, R0, ct2, A2, ALU.add)
            Wrows = [W0, A1, W2]

            # --- m = Ry(-ta2) Rx(-ta1) Rz(-ta0) (c_s - c_t): [P, 3, NB] ---
            GV = T("GV", (P, 3, NB))
            for dd in range(3):
                V.tensor_tensor(GV[:, dd, :], IN_S[:, :, dd], IN_T[:, :, dd], ALU.subtract)
            M3 = T("M3", (P, 3, NB))

            def vec_combine(dst, c, ga, s, gb, op):
                p0, p1 = TW(), TW()
                V.tensor_mul(p0, c, ga)
                V.tensor_mul(p1, s, gb)
                V.tensor_tensor(dst, p0, p1, op)

            g0 = T("g0")
            g1 = T("g1")
            vec_combine(g0, ct0, GV[:, 0, :], st0, GV[:, 1, :], ALU.add)
            vec_combine(g1, ct0, GV[:, 1, :], st0, GV[:, 0, :], ALU.subtract)
            m2a = T("m2a")
            vec_combine(M3[:, 1, :], ct1, g1, st1, GV[:, 2, :], ALU.add)
            vec_combine(m2a, ct1, GV[:, 2, :], st1, g1, ALU.subtract)
            vec_combine(M3[:, 0, :], ct2, g0, st2, m2a, ALU.subtract)
            vec_combine(M3[:, 2, :], st2, g0, ct2, m2a, ALU.add)

            # --- scaled columns: U/Vv/Wv [P,3,NB]; H [P,3,NB] = |ht/2| ---
            hs = [T(f"hs{c}", (P, NB)) for c in range(3)]
            for c in range(3):
                V.tensor_scalar(hs[c], IN_S[:, :, 3 + c], 0.5, None, ALU.mult)
            U = T("U", (P, 3, NB))
            Vv = T("Vv", (P, 3, NB))
            Wv = T("Wv", (P, 3, NB))
            for dd in range(3):
                V.tensor_mul(U[:, dd, :], hs[0], Wrows[dd][:, 0, :])
                V.tensor_mul(Vv[:, dd, :], hs[1], Wrows[dd][:, 1, :])
                V.tensor_mul(Wv[:, dd, :], hs[2], Wrows[dd][:, 2, :])
            H3 = T("H3", (P, 3, NB))
            for dd in range(3):
                A.activation(H3[:, dd, :], IN_T[:, :, 3 + dd], ABSF, scale=0.5)

            # --- corners: EE [P,2,3,NB], FF [P,4,3,NB] ---
            EE = T("EE", (P, 2, 3, NB))
            V.tensor_add(EE[:, 0, :, :], M3, U)
            V.tensor_sub(EE[:, 1, :, :], M3, U)
            FF = T("FF", (P, 4, 3, NB))
            vb = Vv.unsqueeze(1).broadcast_to([P, 2, 3, NB])
            V.tensor_add(FF[:, 0:2, :, :], EE, vb)
            V.tensor_sub(FF[:, 2:4, :, :], EE, vb)

            # --- tail: per f-slice corner pair, (|y| - H)^2 accumulated ---
            acc = T("acc", (P, 4), F32)
            hb = H3.unsqueeze(1).broadcast_to([P, 2, 3, NB])
            wb = Wv.unsqueeze(1)

            def TT(tag):
                return tailp.tile([P, 2, 3, NB], BF16, tag=tag, name=tag)[:]

            for g in range(4):
                ff = FF[:, g, :, :].unsqueeze(1)
                yp = TT("yp")
                V.tensor_add(yp[:, 0:1, :, :], ff, wb)
                V.tensor_sub(yp[:, 1:2, :, :], ff, wb)
                ay = TT("ay")
                if g % 2 == 1:
                    # odd groups: |y| on DVE to unload the ACT-bound tail
                    V.scalar_tensor_tensor(ay, yp, -1.0, yp, ALU.mult, ALU.max)
                else:
                    A.activation(ay, yp, ABSF)
                rr = TT("rr")
                V.tensor_sub(rr, ay, hb)
                sqo = TT("sqo")
                A.activation(sqo, rr, SQ, accum_out=acc[:, g : g + 1])

            # cross-partition sum on the idle PE: ones[128,1]^T @ acc[128,4]
            psum = ctx.enter_context(tc.tile_pool(name="psum", bufs=1, space="PSUM"))
            ones = T("ones", (P, 1), F32)
            V.memset(ones, 1.0)
            red = psum.tile([1, 4], F32, tag="red", name="red")[:]
            nc.tensor.matmul(red, ones, acc)
            part = T("part", (1, 4), F32)
            V.tensor_copy(part, red)
            nc.sync.dma_start(out=outp[:], in_=part)
    nc.finalize()
    return nc


def _get_nc():
    if "nc" not in _CACHE:
        _CACHE["nc"] = _build_nc()
    return _CACHE["nc"]


def _run(in_maps, **kwargs):
    from concourse.bass_utils import run_bass_kernel_spmd

    return run_bass_kernel_spmd(_get_nc(), in_maps, list(range(N_CORES)), **kwargs)


def _make_in_maps(source, target):
    src = np.ascontiguousarray(np.asarray(source, np.float32)).reshape(N_CORES, P * NB, 9)
    tgt = np.ascontiguousarray(np.asarray(target, np.float32)).reshape(N_CORES, P * NB, 9)
    return [{"source": src[c], "target": tgt[c]} for c in range(N_CORES)]


def _loss_from_results(results):
    tot = 0.0
    for r in results:
        tot += float(r["partial"].astype(np.float64).sum())
    return np.float32(tot / (N_TOTAL * 8))


def kernel(source, target):
    res = _run(_make_in_maps(source, target))
    return _loss_from_results(res.results)


# revision 11
# speedup vs baseline: 1.0078x; 1.0078x over previous
"""Trainium2 Bass kernel for the bbox corner-chamfer loss.

Math: the reference builds the 8 corners of each source/target box, forms
the per-box 8x8 squared-distance matrix, takes the min over target corners
and means everything.  In the target box's local frame the target corners
are the axis-aligned set {+-hx} x {+-hy} x {+-hz}, so the min over the 8
target corners separates per coordinate and the 8x8 matrix disappears:

    loss = mean_{n,k} sum_d (|y_k,d| - |h_t,d|/2)^2,   y_k = R_t^T(S_k - c_t)

Mapping (data-parallel over boxes: 8 cores x 128 partitions x 256 lanes,
one [1,4] fp32 partial-sum row per core, host sums and divides by 8N):
  - plane-major packing [P, K, NB]: every per-angle / per-dim scalar is a
    dense [P, NB] plane (innermost stride 1), so ops qualify for the DVE
    2x_1P perf mode
  - bf16 compute for the rotation / corner stages (fp32 trig inputs and
    fp32 accumulation), doubling DVE throughput; the loss is a mean of 2M
    terms so the bf16 rounding bias is ~1e-4 relative
  - the per-partition accumulators are reduced across partitions on the
    otherwise-idle PE (ones^T @ acc) so the output DMA is one descriptor
    instead of 128 four-byte ones (whose completion wait costs ~6us)
"""

import numpy as np

N_TOTAL = 262144
N_CORES = 8
P = 128
NB = N_TOTAL // N_CORES // P  # 256 boxes per partition
PI = float(np.pi)

_CACHE = {}


def _build_nc():
    from contextlib import ExitStack

    from concourse import bacc, tile, mybir

    F32 = mybir.dt.float32
    BF16 = mybir.dt.bfloat16
    ALU = mybir.AluOpType
    SIN = mybir.ActivationFunctionType.Sin
    SQ = mybir.ActivationFunctionType.Square
    ABSF = mybir.ActivationFunctionType.Abs

    nc = bacc.Bacc(None)
    src = nc.declare_dram_parameter("source", [P * NB, 9], F32, isOutput=False)
    tgt = nc.declare_dram_parameter("target", [P * NB, 9], F32, isOutput=False)
    # [1,4] row: a [128,1] output DMA is 128 four-byte descriptors whose
    # completion semaphore takes ~6us; reducing across partitions on the
    # (otherwise idle) PE makes the output DMA a single descriptor
    outp = nc.declare_dram_parameter("partial", [1, 4], F32, isOutput=True)

    with tile.TileContext(nc) as tc:
        with ExitStack() as ctx:
            persist = ctx.enter_context(tc.tile_pool(name="persist", bufs=1))
            work1 = ctx.enter_context(tc.tile_pool(name="work1", bufs=8))
            tailp = ctx.enter_context(tc.tile_pool(name="tailp", bufs=2))

            V, A = nc.vector, nc.scalar

            def T(tag, shape=(P, NB), dt=BF16):
                return persist.tile(list(shape), dt, tag=tag, name=tag)[:]

            def TW(shape=(P, NB)):
                return work1.tile(list(shape), BF16, tag="w1", name="w1")[:]

            IN_S = T("in_s", (P, NB, 9), F32)
            IN_T = T("in_t", (P, NB, 9), F32)
            sv = src[:].rearrange("(p b) f -> p b f", p=P)
            tv = tgt[:].rearrange("(p b) f -> p b f", p=P)
            CH = NB // 4
            for q in range(4):
                sl = slice(q * CH, (q + 1) * CH)
                nc.sync.dma_start(out=IN_S[:, sl, :], in_=sv[:, sl, :])
                nc.sync.dma_start(out=IN_T[:, sl, :], in_=tv[:, sl, :])

            # one tiny op per DMA chunk so no later instruction needs more
            # than one DMA-semaphore wait (ISA wait-slot limits)
            touch = T("touch", (P, 8), F32)
            for q in range(4):
                i0 = q * CH
                V.tensor_tensor(touch[:, 2 * q : 2 * q + 1], IN_S[:, i0 : i0 + 1, 0], IN_S[:, i0 : i0 + 1, 0], ALU.bypass)
                V.tensor_tensor(touch[:, 2 * q + 1 : 2 * q + 2], IN_T[:, i0 : i0 + 1, 0], IN_T[:, i0 : i0 + 1, 0], ALU.bypass)

            # --- trig: sin/cos of (sa0-ta0, sa1, sa2, ta0, ta1, ta2) ---
            # per-angle wraps (strided field in, dense plane out, fp32),
            # then one Sin per 6-angle pack; ACT writes bf16 planes
            d0 = T("d0", (P, NB), F32)
            V.tensor_sub(d0, IN_S[:, :, 6], IN_T[:, :, 6])
            SINP = persist.tile([P, 6, NB], F32, tag="trigin", name="sinp")[:]
            COSP = persist.tile([P, 6, NB], F32, tag="trigin", name="cosp")[:]
            sa12 = IN_S[:, :, 7:9].transpose([0, 2, 1])  # [P, 2, NB]
            ta012 = IN_T[:, :, 6:9].transpose([0, 2, 1])  # [P, 3, NB]
            for dst, shift in ((SINP, 0.0), (COSP, PI / 2)):
                V.add_range_wrap(dst[:, 0, :], d0, shift, PI, 2 * PI)
                V.add_range_wrap(dst[:, 1:3, :], sa12, shift, PI, 2 * PI)
                V.add_range_wrap(dst[:, 3:6, :], ta012, shift, PI, 2 * PI)
            S6 = T("s6", (P, 6, NB))
            C6 = T("c6", (P, 6, NB))
            A.activation(S6, SINP, SIN)
            A.activation(C6, COSP, SIN)

            sd0, ss1, ss2, st0, st1, st2 = (S6[:, i, :] for i in range(6))
            cd0, cs1, cs2, ct0, ct1, ct2 = (C6[:, i, :] for i in range(6))

            # --- R = Rz(d0) Rx(sa1) Ry(sa2): rows [P, 3, NB], true signs ---
            R0 = T("R0", (P, 3, NB))
            R1 = T("R1", (P, 3, NB))
            R2 = T("R2", (P, 3, NB))
            x12 = TW()
            V.tensor_mul(x12, ss1, ss2)
            x1c2 = TW()
            V.tensor_mul(x1c2, ss1, cs2)

            def combine(dst, a0, a1, b0, b1, op):
                # dst = a0*a1 (op) b0*b1
                p0, p1 = TW(), TW()
                V.tensor_mul(p0, a0, a1)
                V.tensor_mul(p1, b0, b1)
                V.tensor_tensor(dst, p0, p1, op)

            combine(R0[:, 0, :], cd0, cs2, sd0, x12, ALU.subtract)
            V.scalar_tensor_tensor(R0[:, 1, :], sd0, -1.0, cs1, ALU.mult, ALU.mult)
            combine(R0[:, 2, :], cd0, ss2, sd0, x1c2, ALU.add)
            combine(R1[:, 0, :], sd0, cs2, cd0, x12, ALU.add)
            V.tensor_mul(R1[:, 1, :], cd0, cs1)
            combine(R1[:, 2, :], sd0, ss2, cd0, x1c2, ALU.subtract)
            V.scalar_tensor_tensor(R2[:, 0, :], cs1, -1.0, ss2, ALU.mult, ALU.mult)
            V.tensor_copy(R2[:, 1, :], ss1)
            V.tensor_mul(R2[:, 2, :], cs1, cs2)

            # --- W = Ry(-ta2) Rx(-ta1) R  (rows [P,3,NB], broadcast trig) ---
            def b3(ap1):  # [P,NB] -> [P,3,NB]: stride-0 middle, dense inner
                return ap1.unsqueeze(1).broadcast_to([P, 3, NB])

            def row_combine(dst, c, ra, s, rb, op):
                p0, p1 = TW((P, 3, NB)), TW((P, 3, NB))
                V.tensor_mul(p0, ra, b3(c))
                V.tensor_mul(p1, rb, b3(s))
                V.tensor_tensor(dst, p0, p1, op)

            A1 = T("A1", (P, 3, NB))
            A2 = T("A2", (P, 3, NB))
            row_combine(A1, ct1, R1, st1, R2, ALU.add)
            row_combine(A2, ct1, R2, st1, R1, ALU.subtract)
            W0 = T("W0", (P, 3, NB))
            W2 = T("W2", (P, 3, NB))
            row_combine(W0, ct2, R0, st2, A2, ALU.subtract)
            row_combine(W2, st2, R0, ct2, A2, ALU.add)
            Wrows = [W0, A1, W2]

            # --- m = Ry(-ta2) Rx(-ta1) Rz(-ta0) (c_s - c_t): [P, 3, NB] ---
            GV = T("GV", (P, 3, NB))
            for dd in range(3):
                V.tensor_tensor(GV[:, dd, :], IN_S[:, :, dd], IN_T[:, :, dd], ALU.subtract)
            M3 = T("M3", (P, 3, NB))

            def vec_combine(dst, c, ga, s, gb, op):
                p0, p1 = TW(), TW()
                V.tensor_mul(p0, c, ga)
                V.tensor_mul(p1, s, gb)
                V.tensor_tensor(dst, p0, p1, op)

            g0 = T("g0")
            g1 = T("g1")
            vec_combine(g0, ct0, GV[:, 0, :], st0, GV[:, 1, :], ALU.add)
            vec_combine(g1, ct0, GV[:, 1, :], st0, GV[:, 0, :], ALU.subtract)
            m2a = T("m2a")
            vec_combine(M3[:, 1, :], ct1, g1, st1, GV[:, 2, :], ALU.add)
            vec_combine(m2a, ct1, GV[:, 2, :], st1, g1, ALU.subtract)
            vec_combine(M3[:, 0, :], ct2, g0, st2, m2a, ALU.subtract)
            vec_combine(M3[:, 2, :], st2, g0, ct2, m2a, ALU.add)

            # --- scaled columns: U/Vv/Wv [P,3,NB]; H [P,3,NB] = |ht/2| ---
            hs = [T(f"hs{c}", (P, NB)) for c in range(3)]
            for c in range(3):
                V.tensor_scalar(hs[c], IN_S[:, :, 3 + c], 0.5, None, ALU.mult)
            U = T("U", (P, 3, NB))
            Vv = T("Vv", (P, 3, NB))
            Wv = T("Wv", (P, 3, NB))
            for dd in range(3):
                V.tensor_mul(U[:, dd, :], hs[0], Wrows[dd][:, 0, :])
                V.tensor_mul(Vv[:, dd, :], hs[1], Wrows[dd][:, 1, :])
                V.tensor_mul(Wv[:, dd, :], hs[2], Wrows[dd][:, 2, :])
            H3 = T("H3", (P, 3, NB))
            for dd in range(3):
                A.activation(H3[:, dd, :], IN_T[:, :, 3 + dd], ABSF, scale=0.5)

            # --- corners: EE [P,2,3,NB], FF [P,4,3,NB] ---
            EE = T("EE", (P, 2, 3, NB))
            V.tensor_add(EE[:, 0, :, :], M3, U)
            V.tensor_sub(EE[:, 1, :, :], M3, U)
            FF = T("FF", (P, 4, 3, NB))
            vb = Vv.unsqueeze(1).broadcast_to([P, 2, 3, NB])
            V.tensor_add(FF[:, 0:2, :, :], EE, vb)
            V.tensor_sub(FF[:, 2:4, :, :], EE, vb)

            # --- tail: per f-slice corner pair, (|y| - H)^2 accumulated ---
            acc = T("acc", (P, 4), F32)
            hb = H3.unsqueeze(1).broadcast_to([P, 2, 3, NB])
            wb = Wv.unsqueeze(1)

            def TT(tag):
                return tailp.tile([P, 2, 3, NB], BF16, tag=tag, name=tag)[:]

            for g in range(4):
                ff = FF[:, g, :, :].unsqueeze(1)
                yp = TT("yp")
                V.tensor_add(yp[:, 0:1, :, :], ff, wb)
                V.tensor_sub(yp[:, 1:2, :, :], ff, wb)
                ay = TT("ay")
                if g % 2 == 1:
                    # odd groups: |y| on DVE to unload the ACT-bound tail
                    V.scalar_tensor_tensor(ay, yp, -1.0, yp, ALU.mult, ALU.max)
                else:
                    A.activation(ay, yp, ABSF)
                rr = TT("rr")
                V.tensor_sub(rr, ay, hb)
                sqo = TT("sqo")
                A.activation(sqo, rr, SQ, accum_out=acc[:, g : g + 1])

            # cross-partition sum on the idle PE: ones[128,1]^T @ acc[128,4]
            psum = ctx.enter_context(tc.tile_pool(name="psum", bufs=1, space="PSUM"))
            ones = T("ones", (P, 1), F32)
            V.memset(ones, 1.0)
            red = psum.tile([1, 4], F32, tag="red", name="red")[:]
            nc.tensor.matmul(red, ones, acc)
            part = T("part", (1, 4), F32)
            V.tensor_copy(part, red)
            nc.sync.dma_start(out=outp[:], in_=part)
    nc.finalize()
    return nc


def _get_nc():
    if "nc" not in _CACHE:
        _CACHE["nc"] = _build_nc()
    return _CACHE["nc"]


def _run(in_maps, **kwargs):
    from concourse.bass_utils import run_bass_kernel_spmd

    return run_bass_kernel_spmd(_get_nc(), in_maps, list(range(N_CORES)), **kwargs)


def _make_in_maps(source, target):
    src = np.ascontiguousarray(np.asarray(source, np.float32)).reshape(N_CORES, P * NB, 9)
    tgt = np.ascontiguousarray(np.asarray(target, np.float32)).reshape(N_CORES, P * NB, 9)
    return [{"source": src[c], "target": tgt[c]} for c in range(N_CORES)]


def _loss_from_results(results):
    tot = 0.0
    for r in results:
        tot += float(r["partial"].astype(np.float64).sum())
    return np.float32(tot / (N_TOTAL * 8))


def kernel(source, target):
    res = _run(_make_in_maps(source, target))
    return _loss_from_results(res.results)
